# revision 26
# baseline (speedup 1.0000x reference)
"""Grouped Conv1d (B=4, T=512, G=129, F=96 -> O=96, K=3, pad=1) on 8 trn2 cores.

Sharding: 129 groups = 16 full groups per core + group 128 split across all
8 cores by (batch b = core//2, T-half = core%2).  SPMD: every core runs the
identical program on its own slice.
"""

from contextlib import ExitStack

import numpy as np

import concourse.bass as bass
import concourse.mybir as mybir
import concourse.tile as tile
from concourse import bacc
from concourse.bass_utils import run_bass_kernel_spmd

B, T, G, F, O, K = 4, 512, 129, 96, 96, 3
NCORES = 8
GPC = 16
NG = GPC + 1
TP = T + 2
TE = T // 2
TEP = TE + 2
GB = 2
NB = GPC // GB


def build_program():
    nc = bacc.Bacc("TRN2", target_bir_lowering=False, debug=False,
                   num_devices=NCORES)

    f32 = mybir.dt.float32
    f16 = mybir.dt.float16

    xm = nc.dram_tensor("xm", [NB, F, GB, B, TP], f16, kind="ExternalInput")
    xe = nc.dram_tensor("xe", [F, TEP], f16, kind="ExternalInput")
    wt = nc.dram_tensor("wt", [F, NG * K * O], f16, kind="ExternalInput")
    bt = nc.dram_tensor("bt", [O, NG], f32, kind="ExternalInput")
    om = nc.dram_tensor("om", [NB, O, GB, B, T], f16, kind="ExternalOutput")
    oe = nc.dram_tensor("oe", [O, TE], f16, kind="ExternalOutput")

    with ExitStack() as ctx:
        tc = ctx.enter_context(tile.TileContext(nc))
        wpool = ctx.enter_context(tc.tile_pool(name="w", bufs=1))
        xpool = ctx.enter_context(tc.tile_pool(name="x", bufs=5))
        opool = ctx.enter_context(tc.tile_pool(name="o", bufs=3))
        pspool = ctx.enter_context(tc.tile_pool(name="ps", bufs=8, space="PSUM"))

        w_sb = wpool.tile([F, NG * K * O], f16)
        b_sb = wpool.tile([O, NG], f32)
        xe_sb = wpool.tile([F, TEP], f16)

        x_tiles = {}

        def load_x(ib, split=False):
            x_sb = xpool.tile([F, GB * B * TP], f16, tag="x", name=f"x{ib}")
            x_tiles[ib] = x_sb
            if not split:
                h = GB * B * TP // 2
                e0 = nc.scalar if ib % 2 == 0 else nc.sync
                e1 = nc.sync if ib % 2 == 0 else nc.scalar
                src = xm[ib].rearrange("f g b t -> f (g b t)")
                e0.dma_start(x_sb[:, :h], src[:, :h])
                e1.dma_start(x_sb[:, h:], src[:, h:])

        def x_piece(ib, u0, u1, eng):
            eng.dma_start(
                x_tiles[ib][:, u0 * TP:u1 * TP],
                xm[ib].rearrange("f g b t -> f (g b t)")[:, u0 * TP:u1 * TP])

        kw = K * O
        load_x(0, split=True)
        load_x(1, split=True)
        nc.sync.dma_start(w_sb[:, :2 * kw], wt[:, :2 * kw])
        x_piece(0, 0, 1, nc.scalar)
        x_piece(0, 4, 6, nc.gpsimd)
        x_piece(0, 1, 2, nc.sync)
        x_piece(0, 2, 4, nc.scalar)
        nc.scalar.dma_start(w_sb[:, 2 * kw:8 * kw],
                            wt[:, 2 * kw:8 * kw])
        x_piece(0, 6, 8, nc.gpsimd)
        nc.scalar.dma_start(b_sb[:], bt[:])
        nc.sync.dma_start(xe_sb[:], xe[:])
        nc.sync.dma_start(w_sb[:, 8 * kw:], wt[:, 8 * kw:])
        x_piece(1, 4, 8, nc.gpsimd)
        x_piece(1, 0, 2, nc.scalar)
        x_piece(1, 2, 4, nc.sync)

        for ib in range(NB):
            if ib + 2 < NB:
                load_x(ib + 2)
            x_sb = x_tiles.pop(ib)
            o_sb = opool.tile([O, GB * B * T], f16, tag="o")
            for j in range(GB):
                i = ib * GB + j
                pss = [pspool.tile([O, T], f32, tag="ps", name=f"ps{b}")
                       for b in range(B)]
                # b-outer / k-inner: each PSUM finishes after 3 consecutive
                # matmuls, so its bias-add copy can start ~1.3us earlier
                # than with k-outer, smoothing the copy/store cadence
                for b in range(B):
                    for k in range(K):
                        nc.tensor.matmul(
                            pss[b][:],
                            w_sb[:, (i * K + k) * O:(i * K + k + 1) * O],
                            x_sb[:, (j * B + b) * TP + k:(j * B + b) * TP + k + T],
                            start=(k == 0),
                            stop=(k == K - 1),
                        )
                    dst = o_sb[:, (j * B + b) * T:(j * B + b + 1) * T]
                    if (j * B + b) % 2 == 0:
                        nc.scalar.add(dst, pss[b][:], b_sb[:, i:i + 1])
                    else:
                        nc.vector.tensor_scalar_add(dst, pss[b][:],
                                                    b_sb[:, i:i + 1])
            om_flat = om[ib].rearrange("o g b t -> o (g b t)")
            st0 = nc.sync if ib % 2 == 0 else nc.scalar
            st1 = nc.scalar if ib % 2 == 0 else nc.sync
            parts = 4 if ib == NB - 1 else 2
            op = GB * B * T // parts
            for p in range(parts):
                eng = st0 if p % 2 == 0 else st1
                eng.dma_start(om_flat[:, p * op:(p + 1) * op],
                              o_sb[:, p * op:(p + 1) * op])

            if ib == 1:
                ps = pspool.tile([O, TE], f32, tag="ps")
                for k in range(K):
                    nc.tensor.matmul(
                        ps[:],
                        w_sb[:, (GPC * K + k) * O:(GPC * K + k + 1) * O],
                        xe_sb[:, k:k + TE],
                        start=(k == 0),
                        stop=(k == K - 1),
                    )
                oe_sb = wpool.tile([O, TE], f16)
                nc.vector.tensor_scalar_add(oe_sb[:], ps[:],
                                            b_sb[:, GPC:GPC + 1])
                nc.sync.dma_start(oe[:], oe_sb[:])

    nc.finalize()
    return nc


def shard_inputs(x, weight, bias):
    x = np.ascontiguousarray(x, dtype=np.float32)
    weight = np.ascontiguousarray(weight, dtype=np.float32)
    bias = np.ascontiguousarray(bias, dtype=np.float32)

    xp = np.pad(x, ((0, 0), (1, 1), (0, 0), (0, 0)))
    xt = xp.transpose(2, 3, 0, 1).astype(np.float16)
    wtr = weight.transpose(2, 0, 3, 1).astype(np.float16)

    in_maps = []
    for c in range(NCORES):
        gs = list(range(c * GPC, (c + 1) * GPC)) + [G - 1]
        b_c, t0 = c // 2, (c % 2) * TE
        xm_c = xt[c * GPC:(c + 1) * GPC].reshape(NB, GB, F, B, TP)
        in_maps.append({
            "xm": np.ascontiguousarray(xm_c.transpose(0, 2, 1, 3, 4)),
            "xe": np.ascontiguousarray(xt[G - 1, :, b_c, t0:t0 + TEP]),
            "wt": np.ascontiguousarray(wtr[:, gs].reshape(F, NG * K * O)),
            "bt": np.ascontiguousarray(bias[gs].T),
            })
    return in_maps


def unshard_outputs(results):
    out = np.empty((B, T, G, O), dtype=np.float32)
    for c in range(NCORES):
        om = results[c]["om"].astype(np.float32)
        om = om.transpose(0, 2, 1, 3, 4).reshape(GPC, O, B, T)
        out[:, :, c * GPC:(c + 1) * GPC, :] = om.transpose(2, 3, 0, 1)
        b_c, t0 = c // 2, (c % 2) * TE
        out[b_c, t0:t0 + TE, G - 1, :] = results[c]["oe"].astype(np.float32).T
    return out


def run(x, weight, bias, **run_kwargs):
    nc = build_program()
    in_maps = shard_inputs(x, weight, bias)
    res = run_bass_kernel_spmd(nc, in_maps, list(range(NCORES)), **run_kwargs)
    return unshard_outputs(res.results), res


def kernel(x, weight, bias):
    out, _ = run(x, weight, bias)
    return out


# revision 28
# speedup vs baseline: 1.1244x; 1.1244x over previous
"""Grouped Conv1d (B=4, T=512, G=129, F=96 -> O=96, K=3, pad=1) on 8 trn2 cores.

Sharding: 129 groups = 16 full groups per core + group 128 split across all
8 cores by (batch b = core//2, T-half = core%2).  SPMD: every core runs the
identical program on its own slice.
"""

from contextlib import ExitStack

import numpy as np

import concourse.bass as bass
import concourse.mybir as mybir
import concourse.tile as tile
from concourse import bacc
from concourse.bass_utils import run_bass_kernel_spmd

B, T, G, F, O, K = 4, 512, 129, 96, 96, 3
NCORES = 8
GPC = 16
NG = GPC + 1
TP = T + 2
TE = T // 2
TEP = TE + 2
GB = 2
NB = GPC // GB


def build_program():
    nc = bacc.Bacc("TRN2", target_bir_lowering=False, debug=False,
                   num_devices=NCORES)

    f32 = mybir.dt.float32
    f16 = mybir.dt.float16

    xm = nc.dram_tensor("xm", [NB, F, GB, B, TP], f16, kind="ExternalInput")
    xe = nc.dram_tensor("xe", [F, TEP], f16, kind="ExternalInput")
    wt = nc.dram_tensor("wt", [F, NG * K * O], f16, kind="ExternalInput")
    bt = nc.dram_tensor("bt", [O, NG], f32, kind="ExternalInput")
    om = nc.dram_tensor("om", [NB, O, GB, B, T], f16, kind="ExternalOutput")
    oe = nc.dram_tensor("oe", [O, TE], f16, kind="ExternalOutput")

    with ExitStack() as ctx:
        tc = ctx.enter_context(tile.TileContext(nc))
        wpool = ctx.enter_context(tc.tile_pool(name="w", bufs=1))
        xpool = ctx.enter_context(tc.tile_pool(name="x", bufs=5))
        opool = ctx.enter_context(tc.tile_pool(name="o", bufs=3))
        pspool = ctx.enter_context(tc.tile_pool(name="ps", bufs=8, space="PSUM"))

        w_sb = wpool.tile([F, NG * K * O], f16)
        b_sb = wpool.tile([O, NG], f32)
        xe_sb = wpool.tile([F, TEP], f16)

        x_tiles = {}

        def load_x(ib, split=False):
            x_sb = xpool.tile([F, GB * B * TP], f16, tag="x", name=f"x{ib}")
            x_tiles[ib] = x_sb
            if not split:
                h = GB * B * TP // 2
                e0 = nc.scalar if ib % 2 == 0 else nc.sync
                e1 = nc.sync if ib % 2 == 0 else nc.scalar
                src = xm[ib].rearrange("f g b t -> f (g b t)")
                e0.dma_start(x_sb[:, :h], src[:, :h])
                e1.dma_start(x_sb[:, h:], src[:, h:])

        def x_piece(ib, u0, u1, eng):
            eng.dma_start(
                x_tiles[ib][:, u0 * TP:u1 * TP],
                xm[ib].rearrange("f g b t -> f (g b t)")[:, u0 * TP:u1 * TP])

        kw = K * O
        load_x(0, split=True)
        load_x(1, split=True)
        # strict compute-need order: w(g0,g1) + x units j0b0, j0b1, j0b2-3,
        # then all of j1, then later weights/tiles.  sync and scalar rings
        # carry the urgent pieces in parallel; gpsimd (SWDGE) takes bulk.
        nc.sync.dma_start(w_sb[:, :2 * kw], wt[:, :2 * kw])      # groups 0-1
        x_piece(0, 0, 1, nc.scalar)                              # unit j0b0
        x_piece(0, 1, 2, nc.sync)                                # unit j0b1
        x_piece(0, 2, 4, nc.scalar)                              # units j0b2-3
        x_piece(0, 4, 8, nc.gpsimd)                              # all of j1
        nc.scalar.dma_start(b_sb[:], bt[:])
        nc.scalar.dma_start(w_sb[:, 2 * kw:8 * kw],              # groups 2-7
                            wt[:, 2 * kw:8 * kw])
        x_piece(1, 0, 4, nc.sync)
        x_piece(1, 4, 8, nc.gpsimd)
        nc.sync.dma_start(w_sb[:, 8 * kw:], wt[:, 8 * kw:])      # groups 8-16
        nc.sync.dma_start(xe_sb[:], xe[:])

        for ib in range(NB):
            if ib + 2 < NB:
                load_x(ib + 2)
            x_sb = x_tiles.pop(ib)
            o_sb = opool.tile([O, GB * B * T], f16, tag="o")
            om_flat = om[ib].rearrange("o g b t -> o (g b t)")
            st0 = nc.sync if ib % 2 == 0 else nc.scalar
            st1 = nc.scalar if ib % 2 == 0 else nc.sync
            last = ib == NB - 1
            for j in range(GB):
                i = ib * GB + j
                pss = [pspool.tile([O, T], f32, tag="ps", name=f"ps{b}")
                       for b in range(B)]
                for k in range(K):
                    for b in range(B):
                        nc.tensor.matmul(
                            pss[b][:],
                            w_sb[:, (i * K + k) * O:(i * K + k + 1) * O],
                            x_sb[:, (j * B + b) * TP + k:(j * B + b) * TP + k + T],
                            start=(k == 0),
                            stop=(k == K - 1),
                        )
                if not (last and j == GB - 1):
                    for b in range(B):
                        dst = o_sb[:, (j * B + b) * T:(j * B + b + 1) * T]
                        if (j * B + b) % 2 == 0:
                            nc.scalar.add(dst, pss[b][:], b_sb[:, i:i + 1])
                        else:
                            nc.vector.tensor_scalar_add(dst, pss[b][:],
                                                        b_sb[:, i:i + 1])
                    if last:
                        # last tile, first group: store now so the rings
                        # only carry the final group afterwards
                        h = B * T // 2
                        for p in range(2):
                            eng = st0 if p == 0 else st1
                            c0 = j * B * T + p * h
                            eng.dma_start(om_flat[:, c0:c0 + h],
                                          o_sb[:, c0:c0 + h])
                else:
                    # very last group: per-batch copy + immediate store so
                    # the end chain after the final matmul is one short copy
                    # and one small store; final unit split across engines
                    for b in range(B):
                        c0 = (j * B + b) * T
                        dst = o_sb[:, c0:c0 + T]
                        if b < B - 1:
                            if b % 2 == 0:
                                nc.scalar.add(dst, pss[b][:], b_sb[:, i:i + 1])
                            else:
                                nc.vector.tensor_scalar_add(
                                    dst, pss[b][:], b_sb[:, i:i + 1])
                            eng = st0 if b % 2 == 0 else st1
                            eng.dma_start(om_flat[:, c0:c0 + T], dst)
                        else:
                            h = T // 2
                            nc.scalar.add(dst[:, :h], pss[b][:, :h],
                                          b_sb[:, i:i + 1])
                            nc.vector.tensor_scalar_add(
                                dst[:, h:], pss[b][:, h:], b_sb[:, i:i + 1])
                            st1.dma_start(om_flat[:, c0:c0 + h], dst[:, :h])
                            st0.dma_start(om_flat[:, c0 + h:c0 + T],
                                          dst[:, h:])
            if not last:
                # store in halves on opposite rings
                for p in range(2):
                    eng = st0 if p == 0 else st1
                    op = GB * B * T // 2
                    eng.dma_start(om_flat[:, p * op:(p + 1) * op],
                                  o_sb[:, p * op:(p + 1) * op])

            if ib == 1:
                ps = pspool.tile([O, TE], f32, tag="ps")
                for k in range(K):
                    nc.tensor.matmul(
                        ps[:],
                        w_sb[:, (GPC * K + k) * O:(GPC * K + k + 1) * O],
                        xe_sb[:, k:k + TE],
                        start=(k == 0),
                        stop=(k == K - 1),
                    )
                oe_sb = wpool.tile([O, TE], f16)
                nc.vector.tensor_scalar_add(oe_sb[:], ps[:],
                                            b_sb[:, GPC:GPC + 1])
                nc.sync.dma_start(oe[:], oe_sb[:])

    nc.finalize()
    return nc


def shard_inputs(x, weight, bias):
    x = np.ascontiguousarray(x, dtype=np.float32)
    weight = np.ascontiguousarray(weight, dtype=np.float32)
    bias = np.ascontiguousarray(bias, dtype=np.float32)

    xp = np.pad(x, ((0, 0), (1, 1), (0, 0), (0, 0)))
    xt = xp.transpose(2, 3, 0, 1).astype(np.float16)
    wtr = weight.transpose(2, 0, 3, 1).astype(np.float16)

    in_maps = []
    for c in range(NCORES):
        gs = list(range(c * GPC, (c + 1) * GPC)) + [G - 1]
        b_c, t0 = c // 2, (c % 2) * TE
        xm_c = xt[c * GPC:(c + 1) * GPC].reshape(NB, GB, F, B, TP)
        in_maps.append({
            "xm": np.ascontiguousarray(xm_c.transpose(0, 2, 1, 3, 4)),
            "xe": np.ascontiguousarray(xt[G - 1, :, b_c, t0:t0 + TEP]),
            "wt": np.ascontiguousarray(wtr[:, gs].reshape(F, NG * K * O)),
            "bt": np.ascontiguousarray(bias[gs].T),
            })
    return in_maps


def unshard_outputs(results):
    out = np.empty((B, T, G, O), dtype=np.float32)
    for c in range(NCORES):
        om = results[c]["om"].astype(np.float32)
        om = om.transpose(0, 2, 1, 3, 4).reshape(GPC, O, B, T)
        out[:, :, c * GPC:(c + 1) * GPC, :] = om.transpose(2, 3, 0, 1)
        b_c, t0 = c // 2, (c % 2) * TE
        out[b_c, t0:t0 + TE, G - 1, :] = results[c]["oe"].astype(np.float32).T
    return out


def run(x, weight, bias, **run_kwargs):
    nc = build_program()
    in_maps = shard_inputs(x, weight, bias)
    res = run_bass_kernel_spmd(nc, in_maps, list(range(NCORES)), **run_kwargs)
    return unshard_outputs(res.results), res


def kernel(x, weight, bias):
    out, _ = run(x, weight, bias)
    return out
